# revision 24
# baseline (speedup 1.0000x reference)
"""Trainium2 Bass kernel for nn_MessageFunction (GNN message passing).

reference:
    edge_out = einsum('ben,em->bmn', e_vw, W_e) + b_e   # [B, 128, N]
    node_out = einsum('bfn,fm->bmn', h_w,  W_n) + b_n   # [B, 128, N]
    out      = relu(concat([edge_out, node_out], axis=1))  # [B, 256, N]

h_v is an unused input (dead in the reference) — never transferred.

Sharding: data-parallel over the node axis (last dim) across 8 cores,
weights/biases replicated. Each core handles 6250 nodes.

The whole pipeline runs in bf16 (inputs, weights, matmul, output), which
halves HBM traffic vs fp32: per-core 12.8 MB in + 12.8 MB out = 25.6 MB
(~72 us at the 358 GB/s per-core HBM roofline). Accumulation stays fp32 in
PSUM; bias+ReLU read PSUM fp32 and round once to bf16 on the way to SBUF.
End-to-end rounding error ~3e-3 rel, comfortably inside the 2e-2 gate.
Host side converts fp32->bf16 (ml_dtypes, round-to-nearest-even) before
upload and upcasts the bf16 result back to fp32.

Per tile:
  - stream [128, ~2k] bf16 tiles of e_vw / h_w per batch,
  - matmul against resident 128x128 bf16 weights (K=128 contraction),
  - bias + ReLU fused: edge half on ScalarE (activation Relu w/ bias),
    node half on VectorE (tensor_scalar add+max) so the two engines
    run in parallel; both write bf16,
  - edge-half store on ACT's HWDGE ring (depends only on ACT's own
    output), node-half store on SWDGE (gpsimd otherwise idle).
"""

import numpy as np
import ml_dtypes

import concourse.bass as bass
import concourse.mybir as mybir
import concourse.tile as tile
from concourse import bacc
from concourse.bass_utils import run_bass_kernel_spmd

N_CORES = 8
B = 4
F = 128      # EDGE_F == NODE_F (contraction dim)
HALF = 128   # output channels per linear
N_NODES = 50000
NS = N_NODES // N_CORES       # 6250 nodes per core
T_MAX = 2176                  # SBUF tile capacity (cols)

# Per-batch tile widths (sum 6250). 4KB+ DMA lines in bf16; the odd 106
# remainder (6250 mod 512) is folded into one 2154-wide tile so no DMA
# line drops below 1KB. Uniform-large tiles: the DMA ramp is limited by
# per-queue transfer pipelining, and small lead-in tiles only slow it.
_BODY = [2048, 2048, 2154]
# batch 0 leads with one 512 tile: its store reaches the write queues
# ~2.5us earlier (one matmul+act after the pinned first-matmul time),
# with minimal dent to the load ramp
_TILES0 = [512, 2048, 2048, 1642]
# last batch tapered at the end: final tiles small so the store drain
# after the last load is short
_TILESL = [2154, 2048, 1024, 512, 512]
# store routing for the last batch: the sync queue is idle once its last
# load is dispatched, so spreading the drain's writes over all three
# queues keeps the write-only tail at full HBM rate. sync-routed stores
# are emitted AFTER all loads (deferred) to avoid head-of-line blocking
# of load dispatches on the sync engine.
_EDGE_QL = ["scalar", "scalar", "sync", "scalar", "sync"]
_NODE_QL = ["sync", "gpsimd", "gpsimd", "sync", "gpsimd"]


def _mm_splits(width):
    # <=512 fp32 accumulators per matmul (one PSUM bank); near-uniform
    n = -(-width // 512)
    base, rem = divmod(width, n)
    return [base + (1 if i < rem else 0) for i in range(n)]

_BF16 = mybir.dt.bfloat16
_FP32 = mybir.dt.float32

_compiled = None


def _build():
    nc = bacc.Bacc(
        "TRN2",
        target_bir_lowering=False,
        debug=False,
        num_devices=N_CORES,
    )
    e_vw = nc.dram_tensor("e_vw", (B, F, NS), _BF16, kind="ExternalInput").ap()
    h_w = nc.dram_tensor("h_w", (B, F, NS), _BF16, kind="ExternalInput").ap()
    # all constants packed into one tensor = one DMA off the critical path
    # to the first matmul: cols [0:128]=W_e, [128:256]=W_n, then b_e and
    # b_n as fp32 bit patterns (2 bf16 slots each; bitcast on device)
    consts = nc.dram_tensor("consts", (F, 2 * HALF + 4), _BF16, kind="ExternalInput").ap()
    out = nc.dram_tensor("out", (B, 2 * HALF, NS), _BF16, kind="ExternalOutput").ap()

    relu = mybir.ActivationFunctionType.Relu
    alu_add = mybir.AluOpType.add
    alu_max = mybir.AluOpType.max

    with tile.TileContext(nc) as tc:
        tiles = []
        for bb in range(B):
            n0 = 0
            for width in (
                _TILES0 if bb == 0 else _TILESL if bb == B - 1 else _BODY
            ):
                tiles.append((bb, n0, width))
                n0 += width

        with (
            tc.tile_pool(name="consts", bufs=1) as cpool,
            tc.tile_pool(name="xin", bufs=10) as inpool,
            tc.tile_pool(name="xout", bufs=6) as outpool,
            tc.tile_pool(name="psum", bufs=8, space="PSUM") as pspool,
        ):
            c_sb = cpool.tile([F, 2 * HALF + 4], _BF16, tag="consts")
            # one const DMA on ACT's HWDGE ring: dispatches in parallel with
            # sync's first loads
            nc.scalar.dma_start(c_sb[:], consts)
            w_e_sb = c_sb[:, 0:HALF]
            w_n_sb = c_sb[:, HALF : 2 * HALF]
            b_e_sb = c_sb[:, 2 * HALF : 2 * HALF + 2].bitcast(_FP32)
            b_n_sb = c_sb[:, 2 * HALF + 2 : 2 * HALF + 4].bitcast(_FP32)

            n_last = len(_TILESL)
            deferred_sync_stores = []
            for idx, (bb, n0, width) in enumerate(tiles):
                last_i = idx - (len(tiles) - n_last)  # >=0 inside last batch
                sl = bass.ds(n0, width)
                e_t = inpool.tile([F, T_MAX], _BF16, tag="e")
                h_t = inpool.tile([F, T_MAX], _BF16, tag="h")
                nc.sync.dma_start(e_t[:, :width], e_vw[bb, :, sl])
                nc.sync.dma_start(h_t[:, :width], h_w[bb, :, sl])
                o_e = outpool.tile([F, T_MAX], _BF16, tag="oe")
                o_n = outpool.tile([F, T_MAX], _BF16, tag="on")
                # all edge matmuls first, then all node matmuls: fewer
                # weight-buffer alternations on PE
                c0 = 0
                for w in _mm_splits(width):
                    ps_e = pspool.tile([HALF, 512], _FP32, tag="ps")
                    nc.tensor.matmul(ps_e[:, :w], w_e_sb, e_t[:, c0 : c0 + w])
                    nc.scalar.activation(
                        o_e[:, c0 : c0 + w],
                        ps_e[:, :w],
                        relu,
                        bias=b_e_sb,
                    )
                    c0 += w
                # edge-half store from ACT's HWDGE ring: depends only on
                # ACT's own output, so no cross-engine HOL
                if last_i >= 0 and _EDGE_QL[last_i] == "sync":
                    deferred_sync_stores.append((out[bb, 0:HALF, sl], o_e[:, :width]))
                else:
                    nc.scalar.dma_start(out[bb, 0:HALF, sl], o_e[:, :width])
                c0 = 0
                for w in _mm_splits(width):
                    ps_n = pspool.tile([HALF, 512], _FP32, tag="ps")
                    nc.tensor.matmul(ps_n[:, :w], w_n_sb, h_t[:, c0 : c0 + w])
                    nc.vector.tensor_scalar(
                        o_n[:, c0 : c0 + w],
                        ps_n[:, :w],
                        b_n_sb,
                        0.0,
                        alu_add,
                        alu_max,
                    )
                    c0 += w
                # node-half store on SWDGE (gpsimd is otherwise idle)
                if last_i >= 0 and _NODE_QL[last_i] == "sync":
                    deferred_sync_stores.append(
                        (out[bb, HALF : 2 * HALF, sl], o_n[:, :width])
                    )
                else:
                    nc.gpsimd.dma_start(out[bb, HALF : 2 * HALF, sl], o_n[:, :width])

            # drain-phase stores on sync, emitted after every load dispatch
            for dst, src in deferred_sync_stores:
                nc.sync.dma_start(dst, src)

    nc.compile()
    return nc


def _get_nc():
    global _compiled
    if _compiled is None:
        _compiled = _build()
    return _compiled


def run(h_w, e_vw, W_e, b_e, W_n, b_n, trace=False, **kwargs):
    nc = _get_nc()
    bf16 = ml_dtypes.bfloat16
    h_w16 = np.asarray(h_w, dtype=np.float32).astype(bf16)
    e_vw16 = np.asarray(e_vw, dtype=np.float32).astype(bf16)
    consts = np.zeros((F, 2 * HALF + 4), dtype=bf16)
    consts[:, 0:HALF] = np.asarray(W_e, dtype=np.float32).astype(bf16)
    consts[:, HALF : 2 * HALF] = np.asarray(W_n, dtype=np.float32).astype(bf16)
    c_u16 = consts.view(np.uint16)
    c_u16[:, 2 * HALF : 2 * HALF + 2] = (
        np.asarray(b_e, dtype=np.float32).view(np.uint16).reshape(HALF, 2)
    )
    c_u16[:, 2 * HALF + 2 : 2 * HALF + 4] = (
        np.asarray(b_n, dtype=np.float32).view(np.uint16).reshape(HALF, 2)
    )

    in_maps = []
    for c in range(N_CORES):
        sl = slice(c * NS, (c + 1) * NS)
        in_maps.append(
            {
                "e_vw": np.ascontiguousarray(e_vw16[:, :, sl]),
                "h_w": np.ascontiguousarray(h_w16[:, :, sl]),
                "consts": consts,
            }
        )
    res = run_bass_kernel_spmd(
        nc, in_maps, core_ids=list(range(N_CORES)), trace=trace, **kwargs
    )
    full16 = np.concatenate([res.results[c]["out"] for c in range(N_CORES)], axis=2)
    return full16.astype(np.float32), res


def kernel(h_v=None, h_w=None, e_vw=None, W_e=None, b_e=None, W_n=None, b_n=None):
    full, _ = run(h_w, e_vw, W_e, b_e, W_n, b_n, trace=False)
    return full
